# revision 1
# baseline (speedup 1.0000x reference)
"""Banded-Toeplitz HRF stack kernel for Trainium2 (8 NeuronCores, data-parallel).

Problem: theta [512,1] -> H [512,400,400] f32 where
  k[b,:] = gamma_pdf(t, 5, theta_b) - 0.167 * gamma_pdf(t, 15, theta_b)   (30 taps)
  H[b, j, i] = k[b, j-i] if 0 <= j-i < 30 else 0

Strategy (per core, 64 batches):
  * One input DMA loads [128, 91] = [theta | t_rev | c1_rev | c2_rev] (theta
    replicated onto 2 partitions per batch; constants identical per partition).
  * ScalarE computes exp(-theta t); VectorE computes theta powers by repeated
    multiply and the gamma-pdf prefactors; the 30 reversed taps (krev) land at
    columns [370,400) and [570,600) of a [128, 800] SBUF buffer S, with
    S[:, 400:432] zeroed.
  * The output band is written as 3 rectangle DMAs, 30 elements (120 B) per
    row, sourced from sliding/constant windows into S.  Cells outside the band
    stay zero because run_bass_kernel_spmd pre-zeroes ExternalOutput buffers
    (donated zero buffers under the PJRT/axon path -- documented, test-covered
    semantics).  Partition q = 2b + h serves rows [200h, 200h+200) of batch b,
    making the flat DRAM offset linear in q (80000 elements per partition).
    Rects read even partitions only (both partitions of a pair hold identical
    taps; even-strided partitions still hit all 16 SBUF ports).

  Rect A rows   0- 28 cols [0,30)      src y = 399 - j + i  (mid step -1)
  Rect B rows  29-199 cols [j-29, j+1) src y = 370 + i      (mid step  0)
  Rect C rows 200-399 cols [j-29, j+1) src y = 570 + i      (mid step  0)

Band bytes written: 64 x 400 x 120 B = ~3.07 MB per core (vs 40.96 MB dense).
25600 descriptors x 120 B across 16 SDMA engines; cost model ~10.7 ns/desc.
A dummy activation on ScalarE pre-loads the Exp LUT during the input DMA, so
the real exp costs ~210 ns instead of ~1.6 us.  CoreSim cost model: ~23.9 us
per core (transfers ~18.5 us, prefix ~3.4 us, tail ~2 us).
"""

import numpy as np

B = 512
T = 400
L = 30
NCORES = 8
BPC = B // NCORES  # 64 batches per core

SW = 800          # S width per partition (elements)
KREV_EVEN = 370   # reversed taps read by rows 0..199
KREV_ODD = 570    # reversed taps read by rows 200..399

_CACHE = {}


def _host_constants():
    """[91] f32 row: [0 | t_rev | c1_rev | c2_rev] (col 0 is theta's slot).

    t matches the reference grid: f32 linspace(0, 30, 30000)[::1000], clipped
    at 1e-8.  c1 = t^5/Gamma(6), c2 = -0.167 * t^15/Gamma(16), reversed so the
    on-device product E*G directly yields krev[q] = k[29-q].
    """
    t = np.linspace(0.0, 30.0, 30000, dtype=np.float32)[::1000]
    t = np.maximum(t, np.float32(1e-8)).astype(np.float64)
    tr = t[::-1].copy()
    c1r = tr**5 / 120.0
    c2r = -0.167 * tr**15 / 1307674368000.0
    return np.concatenate([[0.0], tr, c1r, c2r]).astype(np.float32)


def _in_map(theta_slice):
    row = np.tile(_host_constants(), (128, 1))
    row[:, 0] = np.repeat(theta_slice, 2)
    return {"inp": np.ascontiguousarray(row, dtype=np.float32)}


def _build_nc():
    import concourse.bass as bass
    import concourse.mybir as mybir
    from concourse.ap import AP
    from contextlib import ExitStack

    f32 = mybir.dt.float32
    nc = bass.Bass()

    inp = nc.declare_dram_parameter("inp", [128, 91], f32, isOutput=False)
    out = nc.declare_dram_parameter("H", [BPC, T, T], f32, isOutput=True)
    out_t = out[:].tensor

    ctx = ExitStack()
    nc._kernel_ctx = ctx  # keep SBUF allocations alive for the program

    cst = ctx.enter_context(nc.sbuf_tensor([128, 91], f32))
    p2 = ctx.enter_context(nc.sbuf_tensor([128, 1], f32))
    p4 = ctx.enter_context(nc.sbuf_tensor([128, 1], f32))
    p6 = ctx.enter_context(nc.sbuf_tensor([128, 1], f32))
    p16 = ctx.enter_context(nc.sbuf_tensor([128, 1], f32))
    arg = ctx.enter_context(nc.sbuf_tensor([128, L], f32))
    ee = ctx.enter_context(nc.sbuf_tensor([128, L], f32))
    g1 = ctx.enter_context(nc.sbuf_tensor([128, L], f32))
    gg = ctx.enter_context(nc.sbuf_tensor([128, L], f32))
    zz = ctx.enter_context(nc.sbuf_tensor([128, 1], f32))
    s = ctx.enter_context(nc.sbuf_tensor([128, SW], f32))

    wsem = ctx.enter_context(nc.semaphore("wsem"))
    isem = ctx.enter_context(nc.semaphore("isem"))
    vsem = ctx.enter_context(nc.semaphore("vsem"))
    asem = ctx.enter_context(nc.semaphore("asem"))
    ksem = ctx.enter_context(nc.semaphore("ksem"))
    osem = ctx.enter_context(nc.semaphore("osem"))

    th = cst[:, 0:1]
    tr_c = cst[:, 1:31]
    c1_c = cst[:, 31:61]
    c2_c = cst[:, 61:91]

    s_t = s[:].tensor

    def src_ap(offset, mid_step, mid_count):
        return AP(tensor=s_t, offset=offset,
                  ap=[[2 * SW, 64], [mid_step, mid_count], [1, L]])

    def dst_ap(offset, mid_step, mid_count):
        return AP(tensor=out_t, offset=offset,
                  ap=[[T * T, 64], [mid_step, mid_count], [1, L]])

    # Rect A: rows 0..28, cols [0,30). y = 399 - j + i (needs S[400:429] == 0).
    a_src = src_ap(399, -1, 29)
    a_dst = dst_ap(0, T, 29)
    # Rect B: rows 29..199, cols [j-29, j+1). y = 370 + i: exactly krev.
    b_src = src_ap(KREV_EVEN, 0, 171)
    b_dst = dst_ap(401 * 29 - 29, 401, 171)
    # Rect C: rows 200..399, cols [j-29, j+1). y = 570 + i: exactly krev.
    c_src = src_ap(KREV_ODD, 0, 200)
    c_dst = dst_ap(401 * 200 - 29, 401, 200)

    with nc.Block() as block:

        @block.sync
        def _(sync):
            sync.dma_start(cst[:], inp[:]).then_inc(isem, 16)
            sync.dma_start(b_dst, b_src)._wait_ge(ksem, 3).then_inc(osem, 16)
            sync.dma_start(a_dst, a_src)._wait_ge(ksem, 3).then_inc(osem, 16)
            sync.wait_ge(osem, 48)

        @block.scalar
        def _(scalar):
            # dummy exp to load the ACT Exp table while the input DMA runs
            scalar.wait_ge(wsem, 1)
            scalar.activation(zz[:], zz[:], bass.mybir.ActivationFunctionType.Exp)
            scalar.wait_ge(vsem, 1)
            # E = exp(-theta * t_rev)
            scalar.activation(ee[:], arg[:], bass.mybir.ActivationFunctionType.Exp,
                              scale=-1.0).then_inc(asem, 1)
            scalar.dma_start(c_dst, c_src)._wait_ge(ksem, 3).then_inc(osem, 16)
            scalar.wait_ge(osem, 48)

        @block.vector
        def _(vector):
            vector.memset(zz[:], 0.0).then_inc(wsem, 1)
            # only S[400:432] is ever read as zeros (rect A's right margin)
            vector.memset(s[:, 400:432], 0.0).then_inc(ksem, 1)
            vector.wait_ge(isem, 16)
            # arg = theta * t_rev first, to unblock ScalarE's exp early
            vector.tensor_scalar_mul(arg[:], tr_c, th).then_inc(vsem, 1)
            # theta powers by exact repeated multiply (drains: same-engine RAW)
            vector.tensor_mul(p2[:], th, th)
            vector.drain()
            vector.tensor_mul(p4[:], p2[:], p2[:])
            vector.tensor_scalar(p6[:], p2[:], p2[:, 0:1], p2[:, 0:1],
                                 bass.mybir.AluOpType.mult,
                                 bass.mybir.AluOpType.mult)
            vector.drain()
            vector.tensor_scalar(p16[:], p6[:], p6[:, 0:1], p4[:, 0:1],
                                 bass.mybir.AluOpType.mult,
                                 bass.mybir.AluOpType.mult)
            vector.drain()
            # G = theta^6 * c1_rev - 0.167 * theta^16 * c2_rev (sign in c2)
            vector.tensor_scalar_mul(gg[:], c2_c, p16[:, 0:1])
            vector.drain()
            # gg = (c1 * theta^6) + gg, fused on DVE
            vector.scalar_tensor_tensor(gg[:], c1_c, p6[:, 0:1], gg[:],
                                        bass.mybir.AluOpType.mult,
                                        bass.mybir.AluOpType.add)
            vector.wait_ge(asem, 1)
            vector.drain()
            # krev into both band positions of S
            vector.tensor_mul(s[:, KREV_EVEN:KREV_EVEN + L], gg[:], ee[:]).then_inc(
                ksem, 1)
            vector.tensor_mul(s[:, KREV_ODD:KREV_ODD + L], gg[:], ee[:]).then_inc(
                ksem, 1)

    return nc


def _get_nc():
    if "nc" not in _CACHE:
        _CACHE["nc"] = _build_nc()
    return _CACHE["nc"]


def kernel(theta):
    from concourse.bass_utils import run_bass_kernel_spmd

    theta = np.asarray(theta, dtype=np.float32).reshape(B)
    in_maps = [_in_map(theta[c * BPC:(c + 1) * BPC]) for c in range(NCORES)]
    nc = _get_nc()
    res = run_bass_kernel_spmd(nc, in_maps, list(range(NCORES)))
    return np.concatenate([res.results[i]["H"] for i in range(NCORES)], axis=0)

